# revision 15
# baseline (speedup 1.0000x reference)
"""Trainium2 Bass kernel for nn_ComplexSSMState (complex diagonal SSM scan).

Strategy
--------
Shard over the G=8 MIMO groups: core g handles x[:, :, g, :] (B=4 sequences).

Math: the complex recurrence h_t = a_t h_{t-1} + b_t with
a_t = m_t e^{i th_t} (m_t real in (0,1)) is rotated per 256-step chunk:
with U_t = cumsum(th)_t, g_t := e^{-i U_t} h_t satisfies
    g_t = m_t g_{t-1} + e^{-i U_t} b_t,
two *real* first-order recurrences (real/imag parts decouple) that map
directly onto the hardware `tensor_tensor_scan` (op0=mult, op1=add).
Renorm (period 256) is applied at chunk boundaries, exactly as in the
reference (it is an exact no-op when |h| <= 100, but implemented anyway).

Sign convention: the scalar-engine Sin is valid only on [-pi, pi]; we
compute phase in turns u = U/(2pi), reduce fract = mod(u + 64, 1) and use
st = sin(2pi*fract - pi) = -sin(2pi u),  ct (with +0.25 turn) = -cos(2pi u).
The minus signs are absorbed by scanning ghat = -g (beta_hat = -beta) and
flipping adds/subtracts in the rotate-back, so no explicit negations occur.

Per-core layout: b-pairs on partitions ([b0 n0:64 | b1 n0:64]) for the
scan stage; per-b stacked [h_r; h_i] tiles feed the output projection as
K=128 stationary matmuls producing [t, (y_r|y_i)] tiles directly.
"""

import sys

for _p in ("/opt/trn_rl_repo",):
    if _p not in sys.path:
        sys.path.insert(0, _p)

import numpy as np

import concourse.bass as bass
import concourse.bacc as bacc
import concourse.tile as tile
from concourse import mybir
from concourse.bass_utils import run_bass_kernel_spmd

F32 = mybir.dt.float32
F32R = mybir.dt.float32r
ALU = mybir.AluOpType
ACTF = mybir.ActivationFunctionType

B, S, G, D, N = 4, 2048, 8, 128, 64
L = 256            # chunk length == RENORM_PERIOD
NCHUNK = S // L    # 8
NPAIR = B // 2     # 2 b-pairs per core
TWO_PI = float(2.0 * np.pi)

USE_F32R = True    # single-pass fp32 matmuls (4x faster PE, slightly lower precision)

# wconst column layout ([128, WCOLS])
WB0, WB1, WB2 = 0, 64, 128          # Bw_r.T | Bw_i.T | -Bw_i.T
WDT0, WDT1 = 192, 194               # dt_w[:, :128].T | dt_w[:, 128:].T
WC = 196                            # C-stack [128, 256]
WACOL = 452                         # negA dup | A_phase/2pi dup
WK2PI, WKN2PI, WKPIH, WKEPS = 454, 455, 456, 457  # 2pi | -2pi | pi/2 | 1e-8
WZ = 458                            # 256 zero cols
WCOLS = WZ + L
# sconst column layout ([2, SCOLS])
SSEL = 0                            # selector [2, 128]
SDTB = 128                          # dt_b column
SLO, SHI = 129, 130                 # logit clip bounds (ln(1e-4)-dtb, ln(2)-dtb)
SCOLS = 131


def _mm(nc, out, lhsT, rhs, start, stop):
    if USE_F32R:
        lhsT = lhsT.bitcast(F32R)
        rhs = rhs.bitcast(F32R)
    nc.tensor.matmul(out, lhsT, rhs, start=start, stop=stop)


def build_nc() -> bass.Bass:
    nc = bacc.Bacc()

    xt_d = nc.declare_dram_parameter("xt", [2, B, D, S], F32R, isOutput=False)
    wconst_d = nc.declare_dram_parameter("wconst", [D, WCOLS], F32R, isOutput=False)
    sconst_d = nc.declare_dram_parameter("sconst", [2, SCOLS], F32R, isOutput=False)
    y_d = nc.declare_dram_parameter("y", [B, S, 2 * D], F32, isOutput=True)

    with tile.TileContext(nc) as tc:
        with (
            tc.tile_pool(name="consts", bufs=1) as consts,
            tc.tile_pool(name="carry", bufs=1) as carryp,
            tc.tile_pool(name="work", bufs=3) as work,
            tc.tile_pool(name="ysb", bufs=3) as ysbp,
            tc.tile_pool(name="plog", bufs=1, space="PSUM") as plogp,
            tc.tile_pool(name="pmagph", bufs=2, space="PSUM") as pmagphp,
            tc.tile_pool(name="pb", bufs=2, space="PSUM") as pbp,
            tc.tile_pool(name="sep", bufs=1, space="PSUM") as sepp,
            tc.tile_pool(name="py", bufs=1, space="PSUM") as pyp,
        ):
            # ---- constants: two DMAs total ----
            wk = consts.tile([D, WCOLS], F32R)
            nc.sync.dma_start(out=wk[:], in_=wconst_d[:])
            sk = consts.tile([2, SCOLS], F32R)
            nc.sync.dma_start(out=sk[:], in_=sconst_d[:])

            wb0, wb1, wb2 = wk[:, WB0:WB0 + N], wk[:, WB1:WB1 + N], wk[:, WB2:WB2 + N]
            wdt0, wdt1 = wk[:, WDT0:WDT0 + 2], wk[:, WDT1:WDT1 + 2]
            wc_t = wk[:, WC:WC + 2 * D]
            acol = wk[:, WACOL:WACOL + 2].bitcast(F32)
            c2pi = wk[:, WK2PI:WK2PI + 1].bitcast(F32)
            cn2pi = wk[:, WKN2PI:WKN2PI + 1].bitcast(F32)
            cpih = wk[:, WKPIH:WKPIH + 1].bitcast(F32)
            ceps = wk[:, WKEPS:WKEPS + 1].bitcast(F32)
            zc = wk[:, WZ:WZ + L].bitcast(F32)
            sel = sk[:, SSEL:SSEL + 2 * N]
            dtb = sk[:, SDTB:SDTB + 1].bitcast(F32)
            
            magici = consts.tile([2 * N, 1], mybir.dt.int32)
            nc.vector.memset(magici[:], 0x5f3759df)

            # persistent scan carries (ghat space = -h), one per pair
            carries = []
            for p in range(NPAIR):
                ct_ = carryp.tile([2 * N, 2], F32, tag=f"carry{p}")
                nc.vector.memset(ct_[:], 0.0)
                carries.append(ct_)

            ysb_last = []   # one entry per iteration: last ACT-written ysb tile
            it = 0
            for c in range(NCHUNK):
                t0 = c * L
                for p in range(NPAIR):
                    b0, b1 = 2 * p, 2 * p + 1
                    carry = carries[p]

                    # ---- one DMA: x^T chunks for both components and both b's ----
                    # xall[d, comp, b, t]
                    xall = work.tile([D, 2, 2, L], F32R, tag="xall")
                    for comp in range(2):
                        src = xt_d[comp, b0:b0 + 2, :, t0:t0 + L].rearrange("b d t -> d b t")
                        nc.sync.dma_start(out=xall[:, comp, :, :], in_=src)
                    xr0, xi0 = xall[:, 0, 0, :], xall[:, 1, 0, :]
                    xr1, xi1 = xall[:, 0, 1, :], xall[:, 1, 1, :]

                    # ---- dt logits [2, (b,t)=512], accumulate over components ----
                    plog = plogp.tile([2, 2 * L], F32, tag="plog")
                    _mm(nc, plog[:], wdt0, xall[:, 0, :, :], True, False)
                    _mm(nc, plog[:], wdt1, xall[:, 1, :, :], False, True)

                    # dt = clip(exp(z + dt_b), 1e-4, 2); the clip also rounds to f32r
                    dte = work.tile([2, 2 * L], F32, tag="dte")
                    nc.scalar.activation(out=dte[:], in_=plog[:], func=ACTF.Exp, bias=dtb)
                    dt_sb = work.tile([2, 2 * L], F32R, tag="dt_sb")
                    nc.vector.tensor_scalar(out=dt_sb[:], in0=dte[:], scalar1=1e-4,
                                            scalar2=2.0, op0=ALU.max, op1=ALU.min)

                    # ---- broadcast dt rows across partitions via selector matmuls ----
                    # pmp cols 0:256 = dt_mag bcast, cols 256:512 = dt_phase bcast
                    # all matmul outputs must start at partition 0 (ISA): b1's
                    # broadcast/B-proj land in a scratch tile and are copied up.
                    pmp = pmagphp.tile([2 * N, 2 * L], F32, tag="pmp")
                    sep = sepp.tile([N, 4 * L], F32, tag="sep")
                    _mm(nc, pmp[0:N, 0:L], sel[:, 0:N], dt_sb[:, 0:L], True, True)
                    _mm(nc, pmp[0:N, L:2 * L], sel[:, N:2 * N], dt_sb[:, 0:L], True, True)
                    _mm(nc, sep[:, 0:L], sel[:, 0:N], dt_sb[:, L:2 * L], True, True)
                    _mm(nc, sep[:, L:2 * L], sel[:, N:2 * N], dt_sb[:, L:2 * L], True, True)

                    # ---- B projection (no dt scale): cols 0:256 = b_r, 256:512 = b_i ----
                    pb = pbp.tile([2 * N, 2 * L], F32, tag="pb")
                    _mm(nc, pb[0:N, 0:L], wb0, xr0, True, False)
                    _mm(nc, pb[0:N, 0:L], wb2, xi0, False, True)
                    _mm(nc, sep[:, 2 * L:3 * L], wb0, xr1, True, False)
                    _mm(nc, sep[:, 2 * L:3 * L], wb2, xi1, False, True)
                    _mm(nc, pb[0:N, L:2 * L], wb1, xr0, True, False)
                    _mm(nc, pb[0:N, L:2 * L], wb0, xi0, False, True)
                    _mm(nc, sep[:, 3 * L:4 * L], wb1, xr1, True, False)
                    _mm(nc, sep[:, 3 * L:4 * L], wb0, xi1, False, True)
                    # merge b1 halves up to partitions 64:128
                    nc.scalar.copy(out=pmp[N:2 * N, :], in_=sep[:, 0:2 * L])
                    nc.scalar.copy(out=pb[N:2 * N, :], in_=sep[:, 2 * L:4 * L])

                    # ---- decay magnitude m = exp(dt_mag * -softplus(logAmag)) ----
                    m_t = work.tile([2 * N, L], F32, tag="m_t")
                    nc.scalar.activation(out=m_t[:], in_=pmp[:, 0:L], func=ACTF.Exp,
                                         scale=acol[:, 0:1])
                    # ---- phase increment in turns ----
                    thu = work.tile([2 * N, L], F32, tag="thu")
                    nc.scalar.activation(out=thu[:], in_=pmp[:, L:2 * L], func=ACTF.Copy,
                                         scale=acol[:, 1:2])
                    # cumulative phase u (turns)
                    u_t = work.tile([2 * N, L], F32, tag="u_t")
                    nc.vector.tensor_tensor_scan(out=u_t[:], data0=thu[:], data1=zc,
                                                 initial=0.0, op0=ALU.add, op1=ALU.bypass)
                    # range reduce via round-to-nearest magic: r = u - round(u)
                    MAGIC = 12582912.0  # 1.5 * 2**23
                    rr = work.tile([2 * N, L], F32, tag="rr")
                    ra = work.tile([2 * N, L], F32, tag="ra")
                    nc.vector.tensor_scalar(out=rr[:], in0=u_t[:], scalar1=MAGIC,
                                            scalar2=None, op0=ALU.add)
                    nc.vector.tensor_scalar(out=rr[:], in0=rr[:], scalar1=MAGIC,
                                            scalar2=None, op0=ALU.subtract)
                    nc.vector.tensor_tensor(out=rr[:], in0=u_t[:], in1=rr[:], op=ALU.subtract)
                    nc.vector.tensor_scalar(out=ra[:].bitcast(mybir.dt.int32),
                                            in0=rr[:].bitcast(mybir.dt.int32),
                                            scalar1=0x7fffffff, scalar2=None,
                                            op0=ALU.bitwise_and)
                    # st = sin(2pi u), ct = cos(2pi u) = sin(pi/2 - 2pi|r|)
                    st = work.tile([2 * N, L], F32, tag="st")
                    ctl = work.tile([2 * N, L], F32, tag="ctl")
                    nc.scalar.activation(out=st[:], in_=rr[:], func=ACTF.Sin,
                                         scale=c2pi)
                    nc.scalar.activation(out=ctl[:], in_=ra[:], func=ACTF.Sin,
                                         scale=cn2pi, bias=cpih)

                    # cs = -cos*dt_mag, ss = -sin*dt_mag
                    cs = work.tile([2 * N, L], F32, tag="cs")
                    ss = work.tile([2 * N, L], F32, tag="ss")
                    nc.vector.tensor_tensor(out=cs[:], in0=ctl[:], in1=pmp[:, 0:L], op=ALU.mult)
                    nc.vector.tensor_tensor(out=ss[:], in0=st[:], in1=pmp[:, 0:L], op=ALU.mult)

                    # beta_hat = -e^{-iU} dt b:  bhr = cs*br + ss*bi ; bhi = cs*bi - ss*br
                    tt1 = work.tile([2 * N, L], F32, tag="tt1")
                    tt2 = work.tile([2 * N, L], F32, tag="tt2")
                    bhr = work.tile([2 * N, L], F32, tag="bhr")
                    bhi = work.tile([2 * N, L], F32, tag="bhi")
                    nc.vector.tensor_tensor(out=tt1[:], in0=cs[:], in1=pb[:, 0:L], op=ALU.mult)
                    nc.vector.tensor_tensor(out=tt2[:], in0=ss[:], in1=pb[:, L:2 * L], op=ALU.mult)
                    nc.vector.tensor_tensor(out=bhr[:], in0=tt1[:], in1=tt2[:], op=ALU.add)
                    tt3 = work.tile([2 * N, L], F32, tag="tt3")
                    tt4 = work.tile([2 * N, L], F32, tag="tt4")
                    nc.vector.tensor_tensor(out=tt3[:], in0=cs[:], in1=pb[:, L:2 * L], op=ALU.mult)
                    nc.vector.tensor_tensor(out=tt4[:], in0=ss[:], in1=pb[:, 0:L], op=ALU.mult)
                    nc.vector.tensor_tensor(out=bhi[:], in0=tt3[:], in1=tt4[:], op=ALU.subtract)

                    # ---- the scans: ghat_t = m*ghat_{t-1} + beta_hat ----
                    ghr = work.tile([2 * N, L], F32, tag="ghr")
                    ghi = work.tile([2 * N, L], F32, tag="ghi")
                    nc.vector.tensor_tensor_scan(out=ghr[:], data0=m_t[:], data1=bhr[:],
                                                 initial=carry[:, 0:1], op0=ALU.mult, op1=ALU.add)
                    nc.vector.tensor_tensor_scan(out=ghi[:], data0=m_t[:], data1=bhi[:],
                                                 initial=carry[:, 1:2], op0=ALU.mult, op1=ALU.add)

                    # ---- rotate back: h = e^{iU} g ;  hb = [h_r(n); h_i(n)] per b ----
                    # h_r = ct*ghr - st*ghi ; h_i = ct*ghi + st*ghr (signs already folded)
                    hb0 = work.tile([2 * N, L], F32R, tag="hb0")
                    hb1 = work.tile([2 * N, L], F32R, tag="hb1")
                    rt0 = work.tile([2 * N, L], F32, tag="rt0")
                    rt1 = work.tile([2 * N, L], F32, tag="rt1")
                    # b0 on DVE
                    nc.vector.tensor_tensor(out=hb0[0:N, :], in0=ctl[0:N, :], in1=ghr[0:N, :], op=ALU.mult)
                    nc.vector.tensor_tensor(out=rt0[0:N, :], in0=st[0:N, :], in1=ghi[0:N, :], op=ALU.mult)
                    nc.vector.tensor_tensor(out=hb0[0:N, :], in0=hb0[0:N, :], in1=rt0[0:N, :], op=ALU.subtract)
                    nc.vector.tensor_tensor(out=hb0[N:2 * N, :], in0=ctl[0:N, :], in1=ghi[0:N, :], op=ALU.mult)
                    nc.vector.tensor_tensor(out=rt0[N:2 * N, :], in0=st[0:N, :], in1=ghr[0:N, :], op=ALU.mult)
                    nc.vector.tensor_tensor(out=hb0[N:2 * N, :], in0=hb0[N:2 * N, :], in1=rt0[N:2 * N, :], op=ALU.add)
                    # b1 on GPSIMD (SBUF-only operands)
                    nc.gpsimd.tensor_tensor(out=hb1[0:N, :], in0=ctl[N:2 * N, :], in1=ghr[N:2 * N, :], op=ALU.mult)
                    nc.gpsimd.tensor_tensor(out=rt1[0:N, :], in0=st[N:2 * N, :], in1=ghi[N:2 * N, :], op=ALU.mult)
                    nc.gpsimd.tensor_tensor(out=hb1[0:N, :], in0=hb1[0:N, :], in1=rt1[0:N, :], op=ALU.subtract)
                    nc.gpsimd.tensor_tensor(out=hb1[N:2 * N, :], in0=ctl[N:2 * N, :], in1=ghi[N:2 * N, :], op=ALU.mult)
                    nc.gpsimd.tensor_tensor(out=rt1[N:2 * N, :], in0=st[N:2 * N, :], in1=ghr[N:2 * N, :], op=ALU.mult)
                    nc.gpsimd.tensor_tensor(out=hb1[N:2 * N, :], in0=hb1[N:2 * N, :], in1=rt1[N:2 * N, :], op=ALU.add)

                    # ---- renorm at chunk end (exact no-op unless |h|>100) ----
                    lc = slice(L - 1, L)
                    sq1 = work.tile([2 * N, 1], F32, tag="sq1")
                    sq2 = work.tile([2 * N, 1], F32, tag="sq2")
                    nrm = work.tile([2 * N, 1], F32, tag="nrm")
                    nc.vector.tensor_tensor(out=sq1[:], in0=ghr[:, lc], in1=ghr[:, lc], op=ALU.mult)
                    nc.vector.tensor_tensor(out=sq2[:], in0=ghi[:, lc], in1=ghi[:, lc], op=ALU.mult)
                    nc.vector.tensor_tensor(out=sq1[:], in0=sq1[:], in1=sq2[:], op=ALU.add)
                    # scale = min(1, 100/sqrt(q)) via magic-number rsqrt + 2 Newton
                    # steps (exact no-op unless |h| > 100, matching the reference)
                    I32 = mybir.dt.int32
                    qq = work.tile([2 * N, 1], F32, tag="qq")
                    nc.vector.tensor_scalar(out=qq[:], in0=sq1[:], scalar1=1e-8,
                                            scalar2=None, op0=ALU.add)
                    r0 = work.tile([2 * N, 1], F32, tag="r0")
                    nc.vector.tensor_scalar(out=r0[:].bitcast(I32), in0=qq[:].bitcast(I32),
                                            scalar1=1, scalar2=None, op0=ALU.arith_shift_right)
                    nc.vector.tensor_tensor(out=r0[:].bitcast(I32), in0=magici[:],
                                            in1=r0[:].bitcast(I32), op=ALU.subtract)
                    tn = work.tile([2 * N, 1], F32, tag="tn")
                    for _newton in range(2):
                        nc.vector.tensor_tensor(out=tn[:], in0=qq[:], in1=r0[:], op=ALU.mult)
                        nc.vector.tensor_tensor(out=tn[:], in0=tn[:], in1=r0[:], op=ALU.mult)
                        nc.vector.tensor_scalar(out=tn[:], in0=tn[:], scalar1=-0.5,
                                                scalar2=None, op0=ALU.mult)
                        nc.vector.tensor_scalar(out=tn[:], in0=tn[:], scalar1=1.5,
                                                scalar2=None, op0=ALU.add)
                        nc.vector.tensor_tensor(out=r0[:], in0=r0[:], in1=tn[:], op=ALU.mult)
                    scl = work.tile([2 * N, 1], F32, tag="scl")
                    nc.vector.tensor_scalar(out=scl[:], in0=r0[:], scalar1=100.0,
                                            scalar2=None, op0=ALU.mult)
                    nc.vector.tensor_scalar(out=scl[:], in0=scl[:], scalar1=1.0,
                                            scalar2=None, op0=ALU.min)
                    # per-b duplicated scale columns
                    scl0 = work.tile([2 * N, 1], F32, tag="scl0")
                    scl1 = work.tile([2 * N, 1], F32, tag="scl1")
                    nc.scalar.copy(out=scl0[0:N, :], in_=scl[0:N, :])
                    nc.scalar.copy(out=scl0[N:2 * N, :], in_=scl[0:N, :])
                    nc.scalar.copy(out=scl1[0:N, :], in_=scl[N:2 * N, :])
                    nc.scalar.copy(out=scl1[N:2 * N, :], in_=scl[N:2 * N, :])
                    # fix up stored h at the renorm position
                    nc.vector.tensor_tensor(out=hb0[:, lc], in0=hb0[:, lc], in1=scl0[:], op=ALU.mult)
                    nc.gpsimd.tensor_tensor(out=hb1[:, lc], in0=hb1[:, lc], in1=scl1[:], op=ALU.mult)
                    # carry for next chunk: ghat_init = -h_renorm_end
                    if c + 1 < NCHUNK:
                        t5 = work.tile([2 * N, 1], F32, tag="t5")
                        t6 = work.tile([2 * N, 1], F32, tag="t6")
                        hrl = work.tile([2 * N, 1], F32, tag="hrl")
                        hil = work.tile([2 * N, 1], F32, tag="hil")
                        nc.vector.tensor_tensor(out=t5[:], in0=ctl[:, lc], in1=ghr[:, lc], op=ALU.mult)
                        nc.vector.tensor_tensor(out=t6[:], in0=st[:, lc], in1=ghi[:, lc], op=ALU.mult)
                        nc.vector.tensor_tensor(out=hrl[:], in0=t5[:], in1=t6[:], op=ALU.subtract)
                        nc.vector.tensor_tensor(out=t5[:], in0=ctl[:, lc], in1=ghi[:, lc], op=ALU.mult)
                        nc.vector.tensor_tensor(out=t6[:], in0=st[:, lc], in1=ghr[:, lc], op=ALU.mult)
                        nc.vector.tensor_tensor(out=hil[:], in0=t5[:], in1=t6[:], op=ALU.add)
                        nc.vector.tensor_tensor(out=carry[:, 0:1], in0=hrl[:], in1=scl[:], op=ALU.mult)
                        nc.vector.tensor_tensor(out=carry[:, 1:2], in0=hil[:], in1=scl[:], op=ALU.mult)

                    # ---- output projection: y[t, (y_r|y_i)] = hb^T @ wc ----
                    for bi_, hb in ((b0, hb0), (b1, hb1)):
                        py = pyp.tile([D, 2 * D], F32, tag="py")
                        ysb = ysbp.tile([D, 2 * D], F32, tag="ysb")
                        _mm(nc, py[:], hb[:, 0:D], wc_t, True, True)
                        nc.scalar.copy(out=ysb[:], in_=py[:])
                        nc.scalar.dma_start(out=y_d[bi_, t0:t0 + D, :], in_=ysb[:])
                        py2 = pyp.tile([D, 2 * D], F32, tag="py")
                        ysb2 = ysbp.tile([D, 2 * D], F32, tag="ysb")
                        _mm(nc, py2[:], hb[:, D:2 * D], wc_t, True, True)
                        nc.scalar.copy(out=ysb2[:], in_=py2[:])
                        nc.scalar.dma_start(out=y_d[bi_, t0 + D:t0 + 2 * D, :], in_=ysb2[:])
                    ysb_last.append(ysb2)
                    it += 1

    nc.compile()
    return nc


def make_in_maps(inputs: dict) -> list[dict]:
    x_r = np.asarray(inputs["x_r"], np.float32)
    x_i = np.asarray(inputs["x_i"], np.float32)
    log_A_mag = np.asarray(inputs["log_A_mag"], np.float32)
    A_phase = np.asarray(inputs["A_phase"], np.float32)
    Bw_r = np.asarray(inputs["Bw_r"], np.float32)
    Bw_i = np.asarray(inputs["Bw_i"], np.float32)
    Cw_r = np.asarray(inputs["Cw_r"], np.float32)
    Cw_i = np.asarray(inputs["Cw_i"], np.float32)
    dt_w = np.asarray(inputs["dt_w"], np.float32)
    dt_b = np.asarray(inputs["dt_b"], np.float32)

    neg_log_A = -np.logaddexp(0.0, log_A_mag.astype(np.float64)).astype(np.float32)  # [G,N]
    aph = (A_phase.astype(np.float64) / (2.0 * np.pi)).astype(np.float32)            # [G,N]

    sconst = np.zeros((2, SCOLS), np.float32)
    sconst[0, SSEL:SSEL + N] = 1.0
    sconst[1, SSEL + N:SSEL + 2 * N] = 1.0
    sconst[:, SDTB] = dt_b

    wconst_base = np.zeros((D, WCOLS), np.float32)
    wconst_base[:, WB0:WB0 + N] = Bw_r.T
    wconst_base[:, WB1:WB1 + N] = Bw_i.T
    wconst_base[:, WB2:WB2 + N] = -Bw_i.T
    wconst_base[:, WDT0:WDT0 + 2] = dt_w[:, :D].T
    wconst_base[:, WDT1:WDT1 + 2] = dt_w[:, D:].T
    wconst_base[:, WC:WC + 2 * D] = np.vstack(
        [np.hstack([Cw_r.T, Cw_i.T]), np.hstack([-Cw_i.T, Cw_r.T])]
    )
    wconst_base[:, WK2PI] = TWO_PI
    wconst_base[:, WKN2PI] = -TWO_PI
    wconst_base[:, WKPIH] = np.pi / 2
    wconst_base[:, WKEPS] = 1e-8

    in_maps = []
    for g in range(G):
        xt = np.ascontiguousarray(
            np.stack([x_r[:, :, g, :], x_i[:, :, g, :]]).transpose(0, 1, 3, 2)
        ).astype(np.float32)  # [2,B,D,S]
        wconst = wconst_base.copy()
        wconst[:, WACOL] = np.concatenate([neg_log_A[g], neg_log_A[g]])
        wconst[:, WACOL + 1] = np.concatenate([aph[g], aph[g]])
        in_maps.append(dict(xt=xt, wconst=wconst, sconst=sconst))
    return in_maps


def assemble_output(results: list[dict]) -> np.ndarray:
    y = np.stack([r["y"] for r in results])          # [G, B, S, 2D]
    y = y.reshape(G, B, S, 2, D).transpose(3, 1, 2, 0, 4)  # [2,B,S,G,D]
    return np.ascontiguousarray(y)


_NC_CACHE: list = []


def kernel(**inputs) -> np.ndarray:
    if not _NC_CACHE:
        _NC_CACHE.append(build_nc())
    nc = _NC_CACHE[0]
    in_maps = make_in_maps(inputs)
    res = run_bass_kernel_spmd(nc, in_maps, list(range(G)))
    return assemble_output(res.results)
